# revision 1
# baseline (speedup 1.0000x reference)
"""Contrastive-loss kernel for Trainium2 (8 NeuronCores, Bass/Tile).

Problem: x [32768,128] L2-normed rows, track_idxs [32768] in [0,512),
y [512,8,128] L2-normed. Reference computes S = exp(x @ y_sel.T / 0.3)
with y_sel = y.reshape(4096,128), pos[i,j] = (track_idxs[i] == j % 512),
num = sum(S[pos]), den = sum(S[~pos]), loss = -log(num/(den+1e-9)+1e-10).

Strategy (data-parallel over rows, per the sharding hint):
  - Host: stable-sort rows by track id, shard 4096 rows per core,
    pre-transpose to bf16 [128, 4096] (D on partitions). y replicated
    as bf16 y_sel^T [128, 4096].
  - Device per core: for each 128-row tile, matmul x^T.T @ y^T in bf16
    (PSUM fp32), then ScalarE activation Exp with scale=1/temp and the
    fused accum_out giving per-partition running sums (the "total").
  - Positive-pair sums ("num"): rows in a sorted 128-row tile span only
    a few track ids (window of W tracks starting at t0). A small second
    matmul computes dots against the W*8 candidate positive vectors,
    and an accumulating K=W matmul adds +50*temp to exactly the
    (row, candidate) pairs whose track matches (rank-W one-hot mask,
    prepared on host). Exp with bias=-50 then kills non-matches
    (exp(-50)~0) and leaves matches exact; accum_out gives num sums.
  - Host: sum per-core partials in float64, den = total - num,
    loss = -log(num/(den+1e-9)+1e-10).
"""

import numpy as np
import ml_dtypes

import concourse.bass as bass
import concourse.mybir as mybir
import concourse.tile as tile
from concourse import bacc
from concourse.bass_utils import run_bass_kernel_spmd

# Problem constants (hardcoded per harness contract).
N = 32768
D = 128
T = 512
Q = 8
NCORES = 8
R = N // NCORES            # rows per core = 4096
P = 128                    # partitions
NT = R // P                # row tiles per core = 32
TEMP = 0.3
EPS = 1e-9
EPS2 = 1e-10
SCALE = float(np.float32(1.0) / np.float32(TEMP))
MASK_BUMP = 50.0           # exponent bump for matched pairs
BIAS = -MASK_BUMP
CHUNK = 2048               # main psum chunk (4 banks)
MM_N = 512                 # matmul moving free dim (1 bank)

_CACHE = {}


def _build_program(W):
    """Build the per-core Bass program. W = max tracks spanned by any
    128-row tile (global, so one program serves all cores / SPMD)."""
    W8 = W * Q
    nc = bacc.Bacc("TRN2", target_bir_lowering=False, debug=False,
                   num_devices=NCORES)

    xT_d = nc.dram_tensor("xT", (P, R), mybir.dt.bfloat16,
                          kind="ExternalInput").ap()
    yT_d = nc.dram_tensor("yT", (P, T * Q), mybir.dt.bfloat16,
                          kind="ExternalInput").ap()
    ywinT_d = nc.dram_tensor("ywinT", (P, NT * W8), mybir.dt.bfloat16,
                             kind="ExternalInput").ap()
    a50_d = nc.dram_tensor("a50", (W, R), mybir.dt.bfloat16,
                           kind="ExternalInput").ap()
    bm_d = nc.dram_tensor("bm", (W, W8), mybir.dt.bfloat16,
                          kind="ExternalInput").ap()
    tot_d = nc.dram_tensor("tot", (P, NT * (T * Q // CHUNK)),
                           mybir.dt.float32, kind="ExternalOutput").ap()
    num_d = nc.dram_tensor("num", (P, NT), mybir.dt.float32,
                           kind="ExternalOutput").ap()

    n_chunks = T * Q // CHUNK  # 2

    with tile.TileContext(nc) as tc:
        with (
            tc.tile_pool(name="const", bufs=1) as cp,
            tc.tile_pool(name="sb", bufs=2) as sb,
            tc.tile_pool(name="ps", bufs=2, space="PSUM") as ps,
        ):
            bias_s = cp.tile([P, 1], mybir.dt.float32)
            nc.any.memset(bias_s[:], BIAS)
            xT_s = cp.tile([P, R], mybir.dt.bfloat16)
            yT_s = cp.tile([P, T * Q], mybir.dt.bfloat16)
            ywinT_s = cp.tile([P, NT, W8], mybir.dt.bfloat16)
            a50_s = cp.tile([W, R], mybir.dt.bfloat16)
            bm_s = cp.tile([W, W8], mybir.dt.bfloat16)
            tot_s = cp.tile([P, NT * n_chunks], mybir.dt.float32)
            num_s = cp.tile([P, NT], mybir.dt.float32)

            nc.sync.dma_start(xT_s[:], xT_d)
            nc.sync.dma_start(yT_s[:], yT_d)
            nc.sync.dma_start(ywinT_s[:], ywinT_d.rearrange(
                "p (t w) -> p t w", w=W8))
            nc.sync.dma_start(a50_s[:], a50_d)
            nc.sync.dma_start(bm_s[:], bm_d)

            for r in range(NT):
                lhsT = xT_s[:, r * P:(r + 1) * P]
                for c in range(n_chunks):
                    psm = ps.tile([P, CHUNK], mybir.dt.float32, tag="ps")
                    for m in range(CHUNK // MM_N):
                        col = c * CHUNK + m * MM_N
                        nc.tensor.matmul(
                            psm[:, m * MM_N:(m + 1) * MM_N],
                            lhsT,
                            yT_s[:, col:col + MM_N],
                            start=True, stop=True,
                        )
                    scr = sb.tile([P, CHUNK], mybir.dt.float32, tag="scr")
                    nc.scalar.activation(
                        scr[:], psm[:], mybir.ActivationFunctionType.Exp,
                        scale=SCALE,
                        accum_out=tot_s[:, r * n_chunks + c:
                                        r * n_chunks + c + 1],
                    )
                # num: dots against window candidates + rank-W mask bump
                psn = ps.tile([P, CHUNK], mybir.dt.float32, tag="ps")
                nc.tensor.matmul(
                    psn[:, :W8], lhsT, ywinT_s[:, r],
                    start=True, stop=False,
                )
                nc.tensor.matmul(
                    psn[:, :W8], a50_s[:, r * P:(r + 1) * P], bm_s[:],
                    start=False, stop=True,
                )
                scrn = sb.tile([P, W8], mybir.dt.float32, tag="scrn")
                nc.scalar.activation(
                    scrn[:], psn[:, :W8], mybir.ActivationFunctionType.Exp,
                    scale=SCALE, bias=bias_s[:],
                    accum_out=num_s[:, r:r + 1],
                )

            nc.sync.dma_start(tot_d, tot_s[:])
            nc.sync.dma_start(num_d, num_s[:])

    nc.compile()
    return nc


def prepare_inputs(x, track_idxs, y):
    """Host-side layout prep: sort by track, shard, transpose, cast,
    and build the positive-window tensors. Returns (in_maps, meta)."""
    idx_dtype = track_idxs.dtype
    order = np.argsort(track_idxs, kind="stable")
    xs = np.ascontiguousarray(x[order])
    ts = track_idxs[order].astype(np.int64)

    y_sel = np.ascontiguousarray(y.reshape(T * Q, D))  # j-th row = y[j//Q, j%Q]
    yT = np.ascontiguousarray(y_sel.T).astype(ml_dtypes.bfloat16)

    # Window span per 128-row tile (global max -> uniform SPMD program)
    t_first = ts[0::P]                # first row of each 128-row tile
    t_last = ts[P - 1::P]
    W = int((t_last - t_first).max()) + 1

    W8 = W * Q
    bm = np.zeros((W, W8), np.float32)
    for w in range(W):
        bm[w, w * Q:(w + 1) * Q] = 1.0
    bm = bm.astype(ml_dtypes.bfloat16)

    in_maps = []
    for c in range(NCORES):
        rows = slice(c * R, (c + 1) * R)
        xT = np.ascontiguousarray(xs[rows].T).astype(ml_dtypes.bfloat16)
        tsc = ts[rows]
        ywinT = np.zeros((P, NT, W8), np.float32)
        a50 = np.zeros((W, R), np.float32)
        for r in range(NT):
            t0 = int(tsc[r * P])
            for w in range(W):
                t = t0 + w
                if t >= T:
                    break
                # positives of track t are y_sel columns {t + T*k}
                ywinT[:, r, w * Q:(w + 1) * Q] = y_sel[t::T].T
            seg = tsc[r * P:(r + 1) * P] - t0
            a50[seg, np.arange(r * P, (r + 1) * P)] = MASK_BUMP * TEMP
        in_maps.append({
            "xT": xT,
            "yT": yT,
            "ywinT": np.ascontiguousarray(
                ywinT.reshape(P, NT * W8)).astype(ml_dtypes.bfloat16),
            "a50": a50.astype(ml_dtypes.bfloat16),
            "bm": bm,
        })
    return in_maps, W, idx_dtype


def finalize(results):
    """Combine per-core partials into the scalar loss."""
    num = 0.0
    tot = 0.0
    for res in results:
        num += float(res["num"].astype(np.float64).sum())
        tot += float(res["tot"].astype(np.float64).sum())
    den = tot - num
    loss = -np.log(num / (den + EPS) + EPS2)
    return np.array([loss], dtype=np.float32)


def kernel(x, track_idxs, y):
    x = np.asarray(x)
    track_idxs = np.asarray(track_idxs)
    y = np.asarray(y)
    assert x.shape == (N, D) and y.shape == (T, Q, D)
    # Reference maps y through unique(track_idxs, size=T); with every
    # track present (true for this data) that is the identity.
    assert np.unique(track_idxs).size == T, "kernel assumes all tracks present"

    in_maps, W, _ = prepare_inputs(x, track_idxs, y)
    if W not in _CACHE:
        _CACHE[W] = _build_program(W)
    nc = _CACHE[W]
    res = run_bass_kernel_spmd(nc, in_maps, core_ids=list(range(NCORES)))
    return finalize(res.results)
